# revision 1
# baseline (speedup 1.0000x reference)
"""E8P codebook dequant kernel for 8x TRN2 NeuronCores (Bass/Tile).

Row-parallel sharding: core c handles rows [512c, 512c+512) of weight_q and
produces the matching [512, 11008] f32 slice of the output. grid and scale
are replicated to every core. No cross-core communication.

On-device algorithm (per core):
  - One SBUF table tensor T [128, 32768] f32 holds the scale-folded codebook,
    split across partition halves of each 16-partition GPSIMD group:
      partitions p with p%16 == j < 8:  T[p][s] = scale*grid[s-1][j]
                                        (s in 1..32767 -> entries 0..32766)
      partitions p with p%16 == 8+j:    T[p][s] = scale*grid[32766+s][j]
                                        (s in 1..32767 -> entries 32767..65533)
      slot 0 is 0.0 everywhere (sentinel; ap_gather clamps negative stream
      values to slot 0).
  - T3 [128, 4] covers the two remaining entries: slots 1,2 hold
    scale*grid[65534/65535][j] on low partitions, 0 elsewhere.
  - Index streams (int16, computed in u16 with wraparound then bitcast):
      s1 = idx + 1               valid for idx in [0, 32766]
      s2 = idx - 32766           valid for idx in [32767, 65533]
      s3 = max(idx, 65533) - 65533   -> 0 / 1 / 2
  - Three ap_gather calls per chunk over the shared tables; merge
      out[16g+j] = (X1 + X3)[16g+j] + X2[16g+8+j]
    where exactly one term is nonzero per element, so f32 adds are exact.
"""

import numpy as np

import concourse.bass as bass
import concourse.bacc as bacc
import concourse.tile as tile
import concourse.mybir as mybir
from concourse.bass_utils import run_bass_kernel_spmd

OUT_F = 4096
IN_F = 11008
CODESZ = 8
CB = 65536
N_CORES = 8

ROWS = OUT_F // N_CORES          # 512 rows per core
QCOLS = IN_F // CODESZ           # 1376 codes per row
N_IDX = ROWS * QCOLS             # 704512 indices per core
PER_PART = N_IDX // 128          # 5504 indices per partition (= 4 rows)

F_CHUNK = 344                    # 1376 = 4 * 344: chunks never cross a row
S_CHUNK = F_CHUNK * 16           # 5504 stream elements per group per call
N_CHUNKS = PER_PART // F_CHUNK   # 16
CHUNKS_PER_ROW = QCOLS // F_CHUNK  # 4

_CACHE: dict = {}
REPEAT = 1  # device-work multiplier (timing experiments only)


def _build():
    if "nc" in _CACHE:
        return _CACHE["nc"]
    dt = mybir.dt
    nc = bacc.Bacc("TRN2", target_bir_lowering=False, debug=False,
                   enable_asserts=False, num_devices=N_CORES,
                   dynamic_dma_scratch_size=2048)
    wq_d = nc.dram_tensor("wq", [ROWS, QCOLS], dt.int32, kind="ExternalInput")
    # grid arrives host-transposed [8, 65536] so table loads are contiguous
    grid_d = nc.dram_tensor("gridT", [CODESZ, CB], dt.float32, kind="ExternalInput")
    scale_d = nc.dram_tensor("scale", [1], dt.float32, kind="ExternalInput")
    out_d = nc.dram_tensor("out", [N_CHUNKS * 8 * 8 * F_CHUNK * 16],
                       dt.float32, kind="ExternalOutput")

    with tile.TileContext(nc) as tc:
        with tc.tile_pool(name="tab", bufs=1) as tabp, \
             tc.tile_pool(name="small", bufs=1) as smallp, \
             tc.tile_pool(name="idx", bufs=1) as idxp, \
             tc.tile_pool(name="st", bufs=1) as stp, \
             tc.tile_pool(name="x1", bufs=1) as x1p, \
             tc.tile_pool(name="x3", bufs=1) as x3p, \
             tc.tile_pool(name="xunused", bufs=1) as x2sp:

            # ---- scale broadcast to all 128 partitions ----
            scale_t = smallp.tile([128, 1], dt.float32)
            nc.sync.dma_start(scale_t[:], bass.AP(scale_d, 0, [[0, 128], [1, 1]]))

            # ---- codebook table T ----
            T = tabp.tile([128, 32768], dt.float32)
            for j in range(8):
                # low half: entries 0..32766 -> slots 1..32767
                nc.sync.dma_start(
                    T[:][j::16, 1:32768],
                    bass.AP(grid_d, j * CB, [[0, 8], [1, 32767]]),
                )
                # high half: entries 32767..65533 -> slots 1..32767
                nc.sync.dma_start(
                    T[:][(8 + j)::16, 1:32768],
                    bass.AP(grid_d, j * CB + 32767, [[0, 8], [1, 32767]]),
                )
            nc.vector.memset(T[:][:, 0:1], 0.0)
            # fold scale into the table (f32, same rounding as reference)
            nc.vector.tensor_scalar(T[:], T[:], scale_t[:], None,
                                    mybir.AluOpType.mult)

            # ---- T3 for entries 65534/65535: 768 slots so the mostly-zero
            #      stream spreads reads over 64 addresses (slot-conflict
            #      fix); s3 = (idx&63) + 64*fix -> homeless at 126, 191 ----
            T3 = smallp.tile([128, 192], dt.float32)
            nc.vector.memset(T3[:], 0.0)
            for j in range(8):
                nc.sync.dma_start(
                    T3[:][j::16, 126:127],
                    bass.AP(grid_d, j * CB + 65534, [[0, 8], [1, 1]]),
                )
                nc.sync.dma_start(
                    T3[:][j::16, 191:192],
                    bass.AP(grid_d, j * CB + 65535, [[0, 8], [1, 1]]),
                )
            nc.vector.tensor_scalar(T3[:], T3[:], scale_t[:], None,
                                    mybir.AluOpType.mult)

            add = mybir.AluOpType.add
            sub = mybir.AluOpType.subtract
            mx = mybir.AluOpType.max

            for u in [u for _ in range(REPEAT) for u in range(N_CHUNKS // 2)]:
                ta, tb = 2 * u, 2 * u + 1
                # load both chunks' codes up front for the paired T3 stream
                wq_a = stp.tile([128, F_CHUNK], dt.int32, tag="wqa")
                wq_b = stp.tile([128, F_CHUNK], dt.int32, tag="wqb")
                nc.sync.dma_start(
                    wq_a[:],
                    bass.AP(wq_d, ta * F_CHUNK, [[PER_PART, 128], [1, F_CHUNK]]))
                nc.sync.dma_start(
                    wq_b[:],
                    bass.AP(wq_d, tb * F_CHUNK, [[PER_PART, 128], [1, F_CHUNK]]))
                ida = wq_a[:].bitcast(dt.uint16)[:, 0::2]
                idb = wq_b[:].bitcast(dt.uint16)[:, 0::2]

                # one T3 gather covers both chunks (amortizes call overhead)
                s3p = stp.tile([128, 2 * F_CHUNK], dt.int16, tag="s3p")
                s3b = stp.tile([128, 2 * F_CHUNK], dt.int16, tag="s3b")
                band = mybir.AluOpType.bitwise_and
                shl = mybir.AluOpType.logical_shift_left
                addo = mybir.AluOpType.add
                for (idq, off) in ((ida, 0), (idb, F_CHUNK)):
                    sl = slice(off, off + F_CHUNK)
                    nc.vector.tensor_scalar(
                        s3b[:].bitcast(dt.uint16)[:, sl], idq, 63, None, band)
                    nc.vector.tensor_scalar(
                        s3p[:].bitcast(dt.uint16)[:, sl], idq,
                        65533, 65533, mx, sub)
                nc.vector.tensor_scalar(
                    s3p[:].bitcast(dt.uint16), s3p[:].bitcast(dt.uint16),
                    6, None, shl)
                nc.vector.tensor_tensor(
                    s3p[:].bitcast(dt.uint16), s3p[:].bitcast(dt.uint16),
                    s3b[:].bitcast(dt.uint16), addo)
                X3p = x3p.tile([128, 2 * S_CHUNK], dt.float32)
                nc.gpsimd.ap_gather(X3p[:], T3[:], s3p[:], channels=128,
                                    num_elems=192, d=1, num_idxs=2 * S_CHUNK)

                for (t, idc, x3off) in ((ta, ida, 0), (tb, idb, S_CHUNK)):
                    s12 = stp.tile([128, 2 * F_CHUNK], dt.int16, tag="s12")
                    nc.vector.tensor_scalar(
                        s12[:].bitcast(dt.uint16)[:, 0:F_CHUNK], idc, 1, None, add)
                    nc.vector.tensor_scalar(
                        s12[:].bitcast(dt.uint16)[:, F_CHUNK:], idc, 32766, None, sub)

                    X12 = x1p.tile([128, 2 * S_CHUNK], dt.float32)
                    nc.gpsimd.ap_gather(X12[:], T[:], s12[:], channels=128,
                                        num_elems=32768, d=1, num_idxs=2 * S_CHUNK)
                    X1 = X12[:][:, 0:S_CHUNK]
                    X2 = X12[:][:, S_CHUNK:2 * S_CHUNK]
                    X3c = X3p[:][:, x3off:x3off + S_CHUNK]

                    # in-place partition shift of the high half, then merge
                    shuf = [(8 + i) if (i % 16) < 8 else i for i in range(32)]
                    nc.vector.stream_shuffle(X2, X2, shuf)
                    nc.vector.tensor_add(X3c, X3c, X1)
                    nc.vector.tensor_add(X1, X3c, X2)

                    # ---- planar write back (same layout as before) ----
                    for j in range(8):
                        src_ap = X12[:][j::16, 0:S_CHUNK].rearrange(
                            "p (f pp) -> p f pp", pp=16)
                        blk = 8 * F_CHUNK * 16
                        dst = bass.AP(
                            out_d, (t * 8 + j) * blk,
                            [[F_CHUNK * 16, 8], [16, F_CHUNK], [1, 16]],
                        )
                        nc.sync.dma_start(dst, src_ap)

    nc.compile()
    _CACHE["nc"] = nc
    return nc


def kernel(weight_q: np.ndarray, grid: np.ndarray, scale: np.ndarray) -> np.ndarray:
    weight_q = np.ascontiguousarray(np.asarray(weight_q, dtype=np.int32))
    grid = np.ascontiguousarray(np.asarray(grid, dtype=np.float32))
    scale = np.ascontiguousarray(np.asarray(scale, dtype=np.float32))
    nc = _build()
    grid_t = np.ascontiguousarray(grid.T)   # layout marshalling for replication
    in_maps = []
    for c in range(N_CORES):
        in_maps.append({
            "wq": weight_q[c * ROWS:(c + 1) * ROWS],
            "gridT": grid_t,
            "scale": scale,
        })
    res = run_bass_kernel_spmd(nc, in_maps, core_ids=list(range(N_CORES)))
    shards = []
    for c in range(N_CORES):
        planar = res.results[c]["out"].reshape(N_CHUNKS, 8, 8, F_CHUNK, 16)
        # element (t, j, g, f, pp) -> row 64g + 4pp + t//8,
        #                            col ((t%8)*F_CHUNK + f)*8 + j
        p6 = planar.reshape(4, CHUNKS_PER_ROW, 8, 8, F_CHUNK, 16)  # tt, tq, j, g, f, pp
        # -> [g, pp, tt, tq, f, j]
        x = np.transpose(p6, (3, 5, 0, 1, 4, 2))
        shards.append(x.reshape(ROWS, IN_F))
    return np.concatenate(shards, axis=0)


if __name__ == "__main__":
    rng = np.random.default_rng(0)
    wq = rng.integers(0, CB, size=(OUT_F, QCOLS), dtype=np.int32)
    g = rng.standard_normal((CB, CODESZ)).astype(np.float32)
    s = rng.random(1).astype(np.float32)
    got = kernel(wq, g, s)
    exp = (g[wq].reshape(OUT_F, IN_F) * s).astype(np.float32)
    err = np.abs(got - exp)
    denom = np.maximum(np.abs(exp), 1e-6)
    print("max abs err:", err.max())
    print("max rel err:", (err / denom).max())
    print("exact match:", np.array_equal(got, exp))



# revision 3
# speedup vs baseline: 2.4669x; 2.4669x over previous
"""E8P codebook dequant kernel v3 for 8x TRN2 NeuronCores (Bass/Tile).

Row-parallel: core c handles rows [512c, 512c+512) of weight_q; grid and
scale are folded into a replicated table. No cross-core communication.

Algorithm: ONE ap_gather index per code (vs v1's effective three).
  - Table T [128, 32768 x (lo,hi) fp16 pairs]: partition p holds component
    j = p%8; pair s = (scale*grid[s][j], scale*grid[s+32768][j]). 128KB per
    partition — exactly the ap_gather table limit. Covers the full 65536-
    entry codebook: no sentinel slot, no homeless entries, no fixup pass.
  - Host precomputes idx = code & 0x7FFF (int16, always >= 0 so the ucode
    negative-clamp never fires) and packs bit = code >> 15 into bit-planes.
  - gpsimd ap_gather with fp16 d=2 moves one 4-byte word per index — the
    ucode rescales d to words, so cost equals d=1 f32: measured 27.3ns/idx
    with no per-call overhead when indices are already resident in SBUF.
  - Select lo/hi per element: one DVE bitwise_and expands the bit-plane
    against a broadcast {1,2,4,...,128} pattern (nonzero == pick hi), ACT
    copies the lo lanes, DVE copy_predicated overwrites with hi lanes.
  - Output fp16 (one rounding of the f32 product scale*grid; max rel err
    ~2^-11 << the 2e-2 gate), upcast to f32 on host.

Pipelining notes (all measured on this hardware):
  - DMAs with 128 partition-row descriptors cost ~2-4us per row, so per-
    chunk input DMAs would dominate (~550us each). Instead the whole index
    stream (11KB/partition) and bit-packed masks (10.75KB/partition) are
    preloaded once; per-chunk gathers read SBUF slices.
  - The 16MB table broadcast is split across four engine DMA queues.
  - Out-DMAs (64 partition rows per chunk) are split across two queues so
    their descriptor cost (~130us serial) hides under the ~134us gather.
  - X double-buffered; sel/mskx single (their consumers finish before the
    producers of the next chunk need them).

Layout: gpsimd core a in [0,8) handles rows [64a, 64a+64) of the core's
shard, row-major, as 18 chunks. The 16-partition-shared index stream makes
partitions 16a+j and 16a+8+j compute identical values (per-index cost is
per Q7 core, so the duplicates are free); only partitions with p%16 < 8
are DMA'd out.
"""

import numpy as np

import concourse.bass as bass
import concourse.bacc as bacc
import concourse.tile as tile
import concourse.mybir as mybir
from concourse.bass_utils import run_bass_kernel_spmd

OUT_F = 4096
IN_F = 11008
CODESZ = 8
CB = 65536
N_CORES = 8

ROWS = OUT_F // N_CORES          # 512 rows per core
QCOLS = IN_F // CODESZ           # 1376 codes per row
CODES_G = 64 * QCOLS             # 88064 codes per gpsimd core (64 rows)
PER_PART = CODES_G // 16         # 5504 idx columns per partition
NE = 32768                       # table entries (pairs) per partition

CHUNKS = [4896] * 17 + [4832]    # stream positions per chunk; sum == CODES_G
CI_MAX = max(CHUNKS)

REPEAT = 1                       # device-work multiplier (timing only)
TAB_REPEAT = 1                   # table-load multiplier (timing only)

_CACHE: dict = {}


def _build(n_dev=N_CORES):
    key = (REPEAT, TAB_REPEAT, n_dev)
    if key in _CACHE:
        return _CACHE[key]
    dt = mybir.dt
    assert sum(CHUNKS) == CODES_G and all(c % 16 == 0 for c in CHUNKS)
    nc = bacc.Bacc("TRN2", target_bir_lowering=False, debug=False,
                   enable_asserts=False, num_devices=n_dev)
    tab_d = nc.dram_tensor("tab", [8, 2 * NE], dt.float16, kind="ExternalInput")
    wqi_d = nc.dram_tensor("wqi", [128, PER_PART], dt.int16, kind="ExternalInput")
    bit_d = nc.dram_tensor("bit", [128, CODES_G // 8], dt.uint8,
                           kind="ExternalInput")
    pat_d = nc.dram_tensor("pat", [8], dt.uint8, kind="ExternalInput")
    outa_d = nc.dram_tensor("outa", [32 * CODES_G], dt.float16,
                            kind="ExternalOutput")
    outb_d = nc.dram_tensor("outb", [32 * CODES_G], dt.float16,
                            kind="ExternalOutput")

    qs = [nc.sync, nc.scalar]
    band = mybir.AluOpType.bitwise_and

    with tile.TileContext(nc) as tc:
        with tc.tile_pool(name="tab", bufs=1) as tabp, \
             tc.tile_pool(name="wqi", bufs=1) as wqip, \
             tc.tile_pool(name="bit", bufs=1) as bitp, \
             tc.tile_pool(name="x", bufs=2) as xp, \
             tc.tile_pool(name="sel", bufs=1) as selp, \
             tc.tile_pool(name="msk", bufs=1) as mskp:

            T = tabp.tile([128, 2 * NE], dt.float16)
            W = wqip.tile([128, PER_PART], dt.int16)
            B = bitp.tile([128, CODES_G // 8], dt.uint8)
            P8 = bitp.tile([128, 8], dt.uint8)
            for _ in range(TAB_REPEAT):
                for j in range(8):
                    qs[j % 2].dma_start(
                        T[:][j::8, :],
                        bass.AP(tab_d, j * 2 * NE, [[0, 16], [1, 2 * NE]]))
                nc.gpsimd.dma_start(
                    W[:], bass.AP(wqi_d, 0, [[PER_PART, 128], [1, PER_PART]]))
                nc.gpsimd.dma_start(
                    B[:], bass.AP(bit_d, 0,
                                  [[CODES_G // 8, 128], [1, CODES_G // 8]]))
                nc.sync.dma_start(P8[:], bass.AP(pat_d, 0, [[0, 128], [1, 8]]))

            for _ in range(REPEAT):
                off = 0
                for ci in CHUNKS:
                    X = xp.tile([128, 2 * CI_MAX], dt.float16, tag="x")
                    nc.gpsimd.ap_gather(
                        X[:][:, 0:2 * ci], T[:],
                        W[:][:, off // 16:(off + ci) // 16],
                        channels=128, num_elems=NE, d=2, num_idxs=ci)

                    mskx = mskp.tile([128, CI_MAX], dt.uint8, tag="m")
                    bb = B[:][:, off // 8:(off + ci) // 8]
                    for b in range(8):
                        nc.vector.tensor_scalar(
                            mskx[:][:, b:ci:8], bb, 1 << b, None, band)

                    s = selp.tile([128, CI_MAX], dt.float16, tag="s")
                    nc.scalar.copy(s[:][:, 0:ci], X[:][:, 0:2 * ci:2])
                    nc.vector.copy_predicated(s[:][:, 0:ci], mskx[:][:, 0:ci],
                                              X[:][:, 1:2 * ci:2])

                    for a in range(8):
                        eng = nc.sync if a < 4 else nc.scalar
                        od = outa_d if a < 4 else outb_d
                        eng.dma_start(
                            bass.AP(od, 32 * off + (a % 4) * 8 * ci,
                                    [[ci, 8], [1, ci]]),
                            s[:][16 * a:16 * a + 8, 0:ci])
                    off += ci

    nc.compile()
    _CACHE[key] = nc
    return nc


def prepare_inputs(weight_q, grid, scale):
    """Host marshalling: scale-folded fp16 pair table, 16-wrapped int16 index
    stream, bit-packed hi/lo select planes, bit-position pattern."""
    weight_q = np.ascontiguousarray(np.asarray(weight_q, dtype=np.int32))
    grid = np.ascontiguousarray(np.asarray(grid, dtype=np.float32))
    scale = np.asarray(scale, dtype=np.float32)

    sgf = grid * scale[0]                            # f32 product
    # fp16 subnormals (|v| < 6.1e-5) lose relative precision, so scale the
    # table into the normal range by 2^k and undo it (exactly) on the host:
    # every |v| >= 1e-6 becomes a normal fp16 after scaling with k = 13.
    m = float(np.max(np.abs(sgf))) or 1.0
    k = int(np.clip(np.floor(np.log2(60000.0 / m)), 0, 13))
    sg = (sgf * np.float32(2.0 ** k)).astype(np.float16)
    P = np.empty((8, NE, 2), np.float16)
    P[:, :, 0] = sg[:NE].T
    P[:, :, 1] = sg[NE:].T
    tab = np.ascontiguousarray(P.reshape(8, 2 * NE))
    pat = (1 << np.arange(8, dtype=np.uint8)).astype(np.uint8)

    in_maps = []
    for c in range(N_CORES):
        w = weight_q[c * ROWS:(c + 1) * ROWS].astype(np.uint16)  # [512, 1376]
        idx = (w & 0x7FFF).astype(np.int16).reshape(8, CODES_G)
        bit = (w >> 15).astype(np.uint8).reshape(8, CODES_G)
        wqi = np.ascontiguousarray(
            idx.reshape(8, PER_PART, 16).transpose(0, 2, 1).reshape(128, PER_PART))
        packed = np.packbits(bit, axis=1, bitorder="little")  # [8, CODES_G/8]
        bitp = np.ascontiguousarray(
            np.broadcast_to(packed[:, None, :],
                            (8, 16, CODES_G // 8)).reshape(128, CODES_G // 8))
        in_maps.append({"tab": tab, "wqi": wqi, "bit": bitp, "pat": pat})
    return in_maps, k


def assemble_output(results, k=0):
    shards = []
    for c in range(N_CORES):
        sa, sb = results[c]["outa"], results[c]["outb"]
        off = 0
        parts = []
        for ci in CHUNKS:
            blk = np.concatenate(
                [sa[32 * off:32 * (off + ci)].reshape(4, 8, ci),
                 sb[32 * off:32 * (off + ci)].reshape(4, 8, ci)], axis=0)
            parts.append(blk)
            off += ci
        full = np.concatenate(parts, axis=2)                # [a, j, CODES_G]
        x = full.reshape(8, 8, 64, QCOLS)                   # [a, j, r, q]
        x = np.transpose(x, (0, 2, 3, 1))                   # [a, r, q, j]
        shards.append(x.reshape(ROWS, IN_F))
    full = np.concatenate(shards, axis=0).astype(np.float32)
    return full * np.float32(2.0 ** -k)


def kernel(weight_q: np.ndarray, grid: np.ndarray, scale: np.ndarray) -> np.ndarray:
    nc = _build()
    in_maps, k = prepare_inputs(weight_q, grid, scale)
    res = run_bass_kernel_spmd(nc, in_maps, core_ids=list(range(N_CORES)))
    return assemble_output(res.results, k)


if __name__ == "__main__":
    rng = np.random.default_rng(0)
    wq = rng.integers(0, CB, size=(OUT_F, QCOLS), dtype=np.int32)
    g = rng.standard_normal((CB, CODESZ)).astype(np.float32)
    s = rng.random(1).astype(np.float32)
    got = kernel(wq, g, s)
    exp = (g[wq].reshape(OUT_F, IN_F) * s).astype(np.float32)

    err = np.abs(got - exp)
    denom = np.maximum(np.abs(exp), 1e-6)
    print("max abs err vs f32 ref:", err.max())
    print("max rel err vs f32 ref:", (err / denom).max())
